# revision 13
# baseline (speedup 1.0000x reference)
"""Trainium2 Bass kernel for nn_AttentionModule: full-sequence self-attention.

Reference computation (all fp32):
    x = inputs @ W_proj + b_proj            # [B,4096,256]   (B=4, N=4096)
    q,k,v = x@W_q+b_q, x@W_k+b_k, x@W_v+b_v
    attn = softmax(q @ k^T)                 # [B,4096,4096]
    out  = gamma * (attn @ v) + x

Sharding: 8 cores = 4 batches x 2 query-halves. Core c handles batch
b=c//2, query rows h*2048..h*2048+2048 (h=c%2); keys/values span the
full 4096 sequence of its batch.

Host-side algebra (exact up to fp reassociation):
    q = inputs @ (W_proj W_q) + (b_proj W_q + b_q)       etc.
    gamma folding: gamma*(attn@v) = attn @ (gamma*v), with v's bias
    folded the same way. Softmax denominators come from an extra ones
    column appended to V, so attn is never materialized divided: we
    compute E = exp(scores), C_ext = E @ [V|1], out = C/(rowsum) + x.

Device program per core (float32r matmuls: full PE rate, ~1e-4 rel err):
    inT   [128c, 4096]  <- host-transposed inputs[b]
    qT[g] [128, 2048] = (W_pq[:,g*128:]).T @ inT[:, h*2048:]   g=0,1
    kT[g] [128, 4096]
    v_ext [128t, 32*258] = inT_tile.T @ W_pvg  (+bias, ones col)
    x_sb  [128t, 4096]   = inT_tile.T @ W_proj (+bias)
    for ic in 0..3 (512 queries):                # PSUM: 4 C banks + 2 S banks
      for jt in 0..31 (128 keys):
        S^T psum [128j, 512i] = kT.T @ qT  (2 accumulating matmuls)
        E = exp(S^T) -> SBUF f32r            (ACT, PSUM->SBUF)
        for isub in 0..3: C[isub] += E[:, isub*128:].T @ v_ext[jt]
      epilogue: out = C[:, :256] * recip(C[:,256]) + x_sb -> DMA out
"""

import numpy as np
from contextlib import ExitStack

import concourse.bass as bass
import concourse.tile as tile
from concourse import bacc, mybir
from concourse.bass_utils import run_bass_kernel_spmd

B, SEQ, C_IN, F = 4, 4096, 128, 256
N_CORES = 8
QROWS = SEQ // 2              # queries per core
ICHUNK = 512                  # queries per attention sweep
N_IC = QROWS // ICHUNK        # 4
N_JT = SEQ // 128             # 32 key blocks
VW = F + 2                    # V columns + [ones, pad] (f32r needs even N)
F32, F32R = mybir.dt.float32, mybir.dt.float32r


def build_bass(n_jt=N_JT, n_ic=N_IC, qkv_bufs=2, s_bufs=2, e_bufs=4,
               skip_phase1=False, N_INCHUNK=8, INT_SPLIT_Q=False):
    nc = bacc.Bacc("TRN2", target_bir_lowering=False, debug=False,
                   num_devices=N_CORES)
    d_inT = nc.dram_tensor("inT", [C_IN, SEQ], F32, kind="ExternalInput").ap()
    d_wpq = nc.dram_tensor("w_pq", [C_IN, F], F32, kind="ExternalInput").ap()
    d_wpk = nc.dram_tensor("w_pk", [C_IN, F], F32, kind="ExternalInput").ap()
    d_wpv = nc.dram_tensor("w_pvg", [C_IN, F], F32, kind="ExternalInput").ap()
    d_wp = nc.dram_tensor("w_p", [C_IN, F], F32, kind="ExternalInput").ap()
    d_bqT = nc.dram_tensor("bias_qT", [128, 2], F32, kind="ExternalInput").ap()
    d_bkT = nc.dram_tensor("bias_kT", [128, 2], F32, kind="ExternalInput").ap()
    d_bv = nc.dram_tensor("bias_vg_bc", [128, F], F32, kind="ExternalInput").ap()
    d_bx = nc.dram_tensor("bias_x_bc", [128, F], F32, kind="ExternalInput").ap()
    d_out = nc.dram_tensor("out", [QROWS, F], F32, kind="ExternalOutput").ap()

    with tile.TileContext(nc) as tc, ExitStack() as ctx:
        per = ctx.enter_context(tc.tile_pool(name="per", bufs=1))
        epool = ctx.enter_context(tc.tile_pool(name="epool", bufs=e_bufs))
        opool = ctx.enter_context(tc.tile_pool(name="opool", bufs=4))
        ps_qkv = ctx.enter_context(tc.tile_pool(name="ps_qkv", bufs=qkv_bufs, space="PSUM"))
        ps_s = ctx.enter_context(tc.tile_pool(name="ps_s", bufs=s_bufs, space="PSUM"))
        ps_c = ctx.enter_context(tc.tile_pool(name="ps_c", bufs=4, space="PSUM"))

        # ---- load + round inputs ----------------------------------------
        # Critical path to the first matmul is w_pq + inT chunk 0; issue
        # those first on the HWDGE queue (nc.sync) and push everything else
        # to the SWDGE queue (nc.gpsimd) so they don't serialize ahead.
        wtiles = {}
        for name, dram in [("w_pq", d_wpq), ("w_pk", d_wpk),
                           ("w_pvg", d_wpv), ("w_p", d_wp)]:
            w = per.tile([C_IN, F], F32, tag=name, name=name + "_s")
            wr = per.tile([C_IN, F], F32R, tag=name + "_r", name=name + "_r")
            wtiles[name] = (w, dram, wr)
        wts = {k: v[2] for k, v in wtiles.items()}

        w, dram, wr = wtiles["w_pq"]
        nc.sync.dma_start(w[:], dram[:])
        nc.vector.tensor_copy(wr[:], w[:])

        inT = per.tile([C_IN, SEQ], F32, tag="inT")
        inT_r = per.tile([C_IN, SEQ], F32R, tag="inT_r")
        for s in range(N_INCHUNK):
            w_chunk = SEQ // N_INCHUNK
            sl = bass.ts(s, w_chunk)
            eng = nc.sync if (not INT_SPLIT_Q or s % 2 == 0) else nc.gpsimd
            eng.dma_start(inT[:, sl], d_inT[:, sl])
            nc.vector.tensor_copy(inT_r[:, sl], inT[:, sl])

        for name in ["w_pk", "w_pvg", "w_p"]:
            w, dram, wr = wtiles[name]
            nc.gpsimd.dma_start(w[:], dram[:])
            nc.vector.tensor_copy(wr[:], w[:])

        bqT = per.tile([128, 2], F32, tag="bqT")
        bkT = per.tile([128, 2], F32, tag="bkT")
        bv = per.tile([128, F], F32, tag="bv")
        bx = per.tile([128, F], F32, tag="bx")
        nc.gpsimd.dma_start(bqT[:], d_bqT[:])
        nc.gpsimd.dma_start(bkT[:], d_bkT[:])
        nc.gpsimd.dma_start(bv[:], d_bv[:])
        nc.gpsimd.dma_start(bx[:], d_bx[:])

        # Preload the exp table set during phase 1 (first ACTIVATE of a new
        # set costs ~2.7us for the table DMA; hide it here).
        warm = per.tile([128, 2], F32, tag="warm")
        nc.vector.memset(warm[:], 0.0)
        nc.scalar.activation(warm[:], warm[:],
                             mybir.ActivationFunctionType.Exp)

        # Query rows are inT columns 0..2047: the host rotates the sequence
        # axis so each core's queries come first. Keys/values use all 4096
        # columns; attention is invariant under the simultaneous permutation
        # of keys and V rows, so the rotation leaves results unchanged.

        # ---- qT / kT ----------------------------------------------------
        qT = [per.tile([128, QROWS], F32R, tag=f"qT{g}", name=f"qT{g}") for g in range(2)]
        kT = [per.tile([128, SEQ], F32R, tag=f"kT{g}", name=f"kT{g}") for g in range(2)]
        for g in range(2):
            wq = wts["w_pq"][:, bass.ts(g, 128)]
            wk = wts["w_pk"][:, bass.ts(g, 128)]
            for s in range(QROWS // 512):
                p = ps_qkv.tile([128, 512], F32, tag="ps_qkv")
                nc.tensor.matmul(p[:], wq, inT_r[:, bass.ts(s, 512)],
                                 start=True, stop=True)
                nc.vector.tensor_scalar_add(qT[g][:, bass.ts(s, 512)], p[:],
                                            bqT[:, g:g + 1])
            for s in range(SEQ // 512):
                p = ps_qkv.tile([128, 512], F32, tag="ps_qkv")
                nc.tensor.matmul(p[:], wk, inT_r[:, bass.ts(s, 512)],
                                 start=True, stop=True)
                nc.vector.tensor_scalar_add(kT[g][:, bass.ts(s, 512)], p[:],
                                            bkT[:, g:g + 1])

        # ---- v_ext / x --------------------------------------------------
        v_ext = per.tile([128, N_JT * VW], F32R, tag="v_ext")
        ones_f32 = per.tile([128, 2], F32, tag="ones_f32")
        nc.vector.memset(ones_f32[:], 1.0)
        for jt in range(N_JT):
            p = ps_qkv.tile([128, F], F32, tag="ps_qkv")
            nc.tensor.matmul(p[:], inT_r[:, bass.ts(jt, 128)], wts["w_pvg"][:],
                             start=True, stop=True)
            nc.vector.tensor_add(v_ext[:, jt * VW:jt * VW + F], p[:], bv[:])
            nc.vector.tensor_copy(v_ext[:, jt * VW + F:jt * VW + VW],
                                  ones_f32[:])

        x_sb = per.tile([128, (QROWS // 128) * F], F32, tag="x_sb")
        for it in range(QROWS // 128):
            p = ps_qkv.tile([128, F], F32, tag="ps_qkv")
            nc.tensor.matmul(p[:], inT_r[:, bass.ts(it, 128)], wts["w_p"][:],
                             start=True, stop=True)
            nc.vector.tensor_add(x_sb[:, bass.ts(it, F)], p[:], bx[:])

        # ---- attention --------------------------------------------------
        # Flat software-pipelined loop over t = ic*n_jt + jt. The S^T
        # matmuls for step t+1 are EMITTED before the C matmuls of step t,
        # so PE's in-order queue never head-of-line blocks on exp(t) (ACT):
        # while exp(t) runs, PE executes S(t+1); C(t) follows.
        steps = [(ic, jt) for ic in range(n_ic) for jt in range(n_jt)]
        pcs = {}       # ic -> list of 4 psum C tiles
        es = {}        # t -> (e tile, ps tile)

        def emit_s(t):
            ic, jt = steps[t]
            ps = ps_s.tile([128, ICHUNK], F32, tag="ps_s", name=f"ps{t}")
            isl, jsl = bass.ts(ic, ICHUNK), bass.ts(jt, 128)
            nc.tensor.matmul(ps[:], kT[0][:, jsl], qT[0][:, isl],
                             start=True, stop=False)
            nc.tensor.matmul(ps[:], kT[1][:, jsl], qT[1][:, isl],
                             start=False, stop=True)
            es[t] = ps

        def emit_exp(t):
            ps = es[t]
            e = epool.tile([128, ICHUNK], F32R, tag="e", name=f"e{t}")
            nc.scalar.activation(e[:], ps[:], mybir.ActivationFunctionType.Exp)
            es[t] = e

        def emit_c(t):
            ic, jt = steps[t]
            if jt == 0:
                pcs[ic] = [ps_c.tile([128, VW], F32, tag="ps_c",
                                     name=f"pc{ic}_{i}") for i in range(4)]
            e, vsl = es.pop(t), v_ext[:, steps[t][1] * VW:(steps[t][1] + 1) * VW]
            for isub in range(4):
                nc.tensor.matmul(pcs[ic][isub][:], e[:, bass.ts(isub, 128)],
                                 vsl, start=(jt == 0), stop=(jt == n_jt - 1))

        def emit_epilogue(ic):
            for isub in range(4):
                row = ic * 4 + isub
                recip = opool.tile([128, 1], F32, tag="recip",
                                   name=f"recip{row}")
                nc.vector.reciprocal(recip[:], pcs[ic][isub][:, F:F + 1])
                o = opool.tile([128, F], F32, tag="o", name=f"o{row}")
                nc.vector.tensor_scalar_mul(o[:], pcs[ic][isub][:, 0:F],
                                            recip[:])
                nc.vector.tensor_add(o[:], o[:], x_sb[:, bass.ts(row, F)])
                nc.sync.dma_start(d_out[row * 128:(row + 1) * 128, :], o[:])
            del pcs[ic]

        # Pipeline depth AHEAD: S matmuls for step t+AHEAD are emitted before
        # the C matmuls of step t, so PE's in-order queue has AHEAD S-pairs
        # of slack to cover exp latency. Needs s_bufs >= AHEAD + 1.
        ahead = s_bufs - 1
        nsteps = len(steps)
        for t in range(min(ahead, nsteps)):
            emit_s(t)
            emit_exp(t)
        for t in range(nsteps):
            if t + ahead < nsteps:
                emit_s(t + ahead)
                emit_exp(t + ahead)
            emit_c(t)
            ic, jt = steps[t]
            if jt == n_jt - 1:
                emit_epilogue(ic)

    nc.compile()
    return nc


_NC_CACHE = {}


def get_nc():
    if "nc" not in _NC_CACHE:
        _NC_CACHE["nc"] = build_bass()
    return _NC_CACHE["nc"]


def make_in_maps(inputs, W_proj, b_proj, W_q, b_q, W_k, b_k, W_v, b_v, gamma):
    f64 = np.float64
    Wp, Wq, Wk, Wv = [np.asarray(a, f64) for a in (W_proj, W_q, W_k, W_v)]
    bp, bq, bk, bvv = [np.asarray(a, f64) for a in (b_proj, b_q, b_k, b_v)]
    g = float(np.asarray(gamma, f64).reshape(()))

    w_pq = (Wp @ Wq).astype(np.float32)
    w_pk = (Wp @ Wk).astype(np.float32)
    w_pvg = (g * (Wp @ Wv)).astype(np.float32)
    w_p = np.ascontiguousarray(np.asarray(W_proj, np.float32))
    bias_q = (bp @ Wq + bq).astype(np.float32)          # [256]
    bias_k = (bp @ Wk + bk).astype(np.float32)
    bias_vg = (g * (bp @ Wv + bvv)).astype(np.float32)
    bias_x = np.asarray(b_proj, np.float32)

    bias_qT = np.ascontiguousarray(bias_q.reshape(2, 128).T)   # [128,2]
    bias_kT = np.ascontiguousarray(bias_k.reshape(2, 128).T)
    bias_vg_bc = np.ascontiguousarray(np.broadcast_to(bias_vg, (128, F)))
    bias_x_bc = np.ascontiguousarray(np.broadcast_to(bias_x, (128, F)))

    inp = np.asarray(inputs, np.float32).reshape(B, SEQ, C_IN)
    in_maps = []
    for c in range(N_CORES):
        b, h = divmod(c, 2)
        # rotate so this core's query rows are columns 0..2047 of inT
        rolled = np.roll(inp[b], -h * QROWS, axis=0) if h else inp[b]
        inT = np.ascontiguousarray(rolled.T)                    # [128, 4096]
        in_maps.append({
            "inT": inT, "w_pq": w_pq, "w_pk": w_pk, "w_pvg": w_pvg,
            "w_p": w_p, "bias_qT": bias_qT, "bias_kT": bias_kT,
            "bias_vg_bc": bias_vg_bc, "bias_x_bc": bias_x_bc,
        })
    return in_maps


def kernel(inputs, W_proj, b_proj, W_q, b_q, W_k, b_k, W_v, b_v, gamma):
    nc = get_nc()
    in_maps = make_in_maps(inputs, W_proj, b_proj, W_q, b_q,
                           W_k, b_k, W_v, b_v, gamma)
    res = run_bass_kernel_spmd(nc, in_maps, core_ids=list(range(N_CORES)))
    out = np.empty((B, SEQ, F), np.float32)
    for c in range(N_CORES):
        b, h = divmod(c, 2)
        out[b, h * QROWS:(h + 1) * QROWS] = res.results[c]["out"]
    return out.reshape(B, 64, 64, F)


if __name__ == "__main__":
    rng = np.random.default_rng(0)
    ins = {
        "inputs": rng.standard_normal((B, 64, 64, C_IN)).astype(np.float32),
        "W_proj": (rng.standard_normal((C_IN, F)) * 0.02).astype(np.float32),
        "b_proj": np.zeros(F, np.float32),
        "W_q": (rng.standard_normal((F, F)) * 0.02).astype(np.float32),
        "b_q": np.zeros(F, np.float32),
        "W_k": (rng.standard_normal((F, F)) * 0.02).astype(np.float32),
        "b_k": np.zeros(F, np.float32),
        "W_v": (rng.standard_normal((F, F)) * 0.02).astype(np.float32),
        "b_v": np.zeros(F, np.float32),
        "gamma": np.array([0.7], np.float32),
    }
    out = kernel(**ins)
    print("out", out.shape, out.dtype, float(np.abs(out).mean()))
